# revision 1
# baseline (speedup 1.0000x reference)
"""Causal self-attention (B=2, T=2048, E=2048, H=16, D=128) on 8 TRN2 cores.

Sharding: core c handles batch b = c//4 and head group g = c%4 (4 heads).
 - W_qkv is split column-wise (per head group) -> each core projects only its
   heads' q/k/v. q,k are produced in [d, t] layout, v in [t, d] layout.
 - RoPE is applied on-device in a half-split basis (host permutes W rows for
   q/k so that rotation pairs are (i, i+64) instead of (2i, 2i+1); scores are
   invariant because q and k get the same permutation).
 - scores are computed transposed (S^T tiles [t_k=128, t_q=512]); softmax is
   max-free (scores*scale stay in [-10, 10] for this problem) with the
   denominator accumulated by a ones-matmul on the PE, and the 1/denom scale
   applied to the attention output before the output projection.
 - W_proj is split row-wise; each core emits a partial [T, E] output and the
   host sums the 4 partials per batch.

All matmuls run in bf16 (fp32 PSUM accumulation). Measured end-to-end
absmax-relative error vs the fp32 reference is ~4e-3.
"""

import math
import os

import numpy as np
import ml_dtypes

import concourse.bass as bass
import concourse.mybir as mybir
import concourse.tile as tile
from concourse.vector_clock import ScopedClock

BF16 = mybir.dt.bfloat16
F32 = mybir.dt.float32

B, T, E = 2, 2048, 2048
H, D = 16, 128
N_CORES = 8
HEADS_PER_CORE = 4
KT = 128          # t_k tile (partitions of score tiles)
QS = 512          # t_q strip (free dim of score tiles)
NKC = E // 128    # contraction chunks for the projections
SCALE = 1.0 / math.sqrt(D)

# The walrus build in this container encodes only one sync-wait command per
# instruction (a 3-wait Drain and a 3-wait TensorTensor both fail codegen
# with "Too many sync wait commands"). Tile's scheduler emits multi-wait
# instructions freely, so after scheduling we move each excess wait onto its
# own preceding same-engine NoOp.
_MAX_WAITS = 1


def _split_excess_waits(nc, max_waits=_MAX_WAITS):
    f = nc.m.functions[0]

    def overloaded(ins):
        si = ins.sync_info
        return si is not None and len(si.on_wait) > max_waits

    plan = {}  # bb name -> set of overloaded instruction names
    for bb in f.blocks:
        names = {ins.name for ins in bb.instructions if overloaded(ins)}
        if names:
            plan[bb.name] = names
    if not plan:
        return

    nop_map = {}   # overloaded inst name -> [nop instruction, ...]
    nop_names = set()
    for bb in f.blocks:
        todo = plan.get(bb.name)
        if not todo:
            continue
        for ins in bb.instructions:
            if ins.name not in todo:
                continue
            si = ins.sync_info
            waits = list(si.on_wait)
            excess, keep = waits[:-max_waits], waits[-max_waits:]
            nops = []
            for w in excess:
                nop = nc.engines[ins.engine].nop(nofuse=True).ins
                nop.sync_info = mybir.SyncInfo(on_wait=[w], on_update=[])
                nops.append(nop)
                nop_names.add(nop.name)
            ins.sync_info = mybir.SyncInfo(
                on_wait=keep, on_update=list(si.on_update))
            nop_map[ins.name] = nops

    # The nop builder appended the new instructions to the tail block;
    # remove them from wherever they landed, then splice each in front of
    # its target instruction (same engine => preserves engine order).
    for bb in f.blocks:
        lst = [i for i in bb.instructions if i.name not in nop_names]
        if bb.name in plan:
            out = []
            for i in lst:
                out.extend(nop_map.get(i.name, ()))
                out.append(i)
            lst = out
        bb.instructions = lst


class _TileContext(tile.TileContext):
    def _drain_and_barrier(self, tick_clock, wait_clock):
        nc = self.nc
        drain_inst = nc.sync.drain()
        wait_clock.add_sem_waits(
            drain_inst.ins, ScopedClock({None: tick_clock.global_clock})
        )
        nc.all_engine_barrier()
        assert self.sems is not None
        popped = nc._tile_sem_poison_stack.pop()
        assert popped is self._sem_poison
        nc.clear_and_free_semaphores(list(self.sems.allocated().values()))
        nc.all_engine_barrier()
        _split_excess_waits(nc)


DEFAULT_CFG = dict(
    ps_a=2, ps_s=3, ps_y=2, ps_d=1,
    expp=6, rope=2, xsp=2, outp=3,
    denom_on_pe=True,   # False: DVE accumulate + gpsimd partition_all_reduce
    v_copy_engine="scalar",
    exp_pair=False,     # one ACT exp over two score tiles (2-bank PSUM)
    rope_from_sbuf=False,  # crossed RoPE reads direct from PSUM (walrus-legal)
    split_dmas=True,      # chunk weight/x DMAs so first matmuls start early
    y_defer_scale=True,   # copy psy out unscaled; apply 1/denom in place
    fuse_proj=True,       # emit the proj for each q-strip right after it
    wp_own_slot=True,     # load W_proj into its own tile at kernel start
)


def build_program(cfg=None, n_iters=1):
    cfg = {**DEFAULT_CFG, **(cfg or {})}
    # the gpsimd partition_all_reduce ucode op doesn't compile on this
    # toolchain; only the PE ones-matmul denominator path is supported
    assert cfg["denom_on_pe"], "denom_on_pe=False requires partition_all_reduce"
    nc = bass.Bass("TRN2", target_bir_lowering=False, debug=False,
                   num_devices=N_CORES)

    xT_d = nc.dram_tensor("xT", [E, T], BF16, kind="ExternalInput")
    wqk_d = nc.dram_tensor("wqk", [E, 1024], BF16, kind="ExternalInput")
    wv_d = nc.dram_tensor("wv", [E, 512], BF16, kind="ExternalInput")
    wp_d = nc.dram_tensor("wp", [512, E], BF16, kind="ExternalInput")
    cos_d = nc.dram_tensor("cos", [128, T], F32, kind="ExternalInput")
    sin_d = nc.dram_tensor("sin", [128, T], F32, kind="ExternalInput")
    out_d = nc.dram_tensor("out", [T, E], F32, kind="ExternalOutput")

    from contextlib import ExitStack

    with _TileContext(nc) as tc, ExitStack() as ctx:
        consts = ctx.enter_context(tc.tile_pool(name="consts", bufs=1))
        wshare = ctx.enter_context(tc.tile_pool(name="wshare", bufs=1))
        xsp = ctx.enter_context(tc.tile_pool(name="xsp", bufs=cfg["xsp"]))
        qkp = ctx.enter_context(tc.tile_pool(name="qkp", bufs=1))
        vp = ctx.enter_context(tc.tile_pool(name="vp", bufs=1))
        yp = ctx.enter_context(tc.tile_pool(name="yp", bufs=1))
        rope = ctx.enter_context(tc.tile_pool(name="rope", bufs=cfg["rope"]))
        expp = ctx.enter_context(tc.tile_pool(name="expp", bufs=cfg["expp"]))
        denp = ctx.enter_context(tc.tile_pool(name="denp", bufs=cfg.get("denp", 2)))
        outp = ctx.enter_context(tc.tile_pool(name="outp", bufs=cfg["outp"]))
        ps_a = ctx.enter_context(
            tc.tile_pool(name="ps_a", bufs=cfg["ps_a"], space="PSUM"))
        ps_s = ctx.enter_context(
            tc.tile_pool(name="ps_s", bufs=cfg["ps_s"], space="PSUM"))
        ps_y = ctx.enter_context(
            tc.tile_pool(name="ps_y", bufs=cfg["ps_y"], space="PSUM"))
        ps_d = None
        if cfg["denom_on_pe"]:
            ps_d = ctx.enter_context(
                tc.tile_pool(name="ps_d", bufs=cfg["ps_d"], space="PSUM"))

        dramp = ctx.enter_context(
            tc.tile_pool(name="dramp", bufs=cfg.get("dramp", 4), space="DRAM"))
        for _it in range(n_iters):
            # ---- constants / weights ----
            wqk_re = wqk_d.ap().rearrange("(kc p) f -> p kc f", p=128)
            wqk_sb = wshare.tile([128, NKC, 1024], BF16, tag="w")
            cos_sb = consts.tile([128, T], F32)
            sin_sb = consts.tile([128, T], F32)
            wv_sb = consts.tile([128, NKC, 512], BF16)

            def load_xs(s):
                xs = xsp.tile([128, NKC, QS], BF16, tag="xs")
                src = xT_d.ap()[:, s * QS:(s + 1) * QS].rearrange(
                    "(kc p) t -> p kc t", p=128)
                if cfg["split_dmas"]:
                    for j in range(4):
                        nc.sync.dma_start(out=xs[:, 4 * j:4 * j + 4, :],
                                          in_=src[:, 4 * j:4 * j + 4, :])
                else:
                    nc.sync.dma_start(out=xs, in_=src)
                return xs

            if cfg["split_dmas"]:
                # loads in exact consumption order: the first M-tile group
                # consumes (wqk[:, kc, 0:128], xs[:, kc, :]) for kc = 0..15,
                # so interleave 4-kc chunks of both streams
                xs_next = xsp.tile([128, NKC, QS], BF16, tag="xs")
                xs0_src = xT_d.ap()[:, 0:QS].rearrange(
                    "(kc p) t -> p kc t", p=128)
                xq = nc.gpsimd if cfg.get("xs0_gpsimd", False) else nc.sync
                for j in range(4):
                    kcs = slice(4 * j, 4 * j + 4)
                    nc.sync.dma_start(out=wqk_sb[:, kcs, 0:128],
                                      in_=wqk_re[:, kcs, 0:128])
                    xq.dma_start(out=xs_next[:, kcs, :],
                                 in_=xs0_src[:, kcs, :])
                n_early = cfg.get("wqk_early", 1)
                for m in range(1, 1 + n_early):
                    nc.sync.dma_start(
                        out=wqk_sb[:, :, m * 128:(m + 1) * 128],
                        in_=wqk_re[:, :, m * 128:(m + 1) * 128])
                nc.sync.dma_start(out=cos_sb, in_=cos_d.ap())
                nc.sync.dma_start(out=sin_sb, in_=sin_d.ap())
                for m in range(1 + n_early, 8):
                    nc.sync.dma_start(
                        out=wqk_sb[:, :, m * 128:(m + 1) * 128],
                        in_=wqk_re[:, :, m * 128:(m + 1) * 128])
            else:
                nc.sync.dma_start(out=wqk_sb, in_=wqk_re)
                xs_next = load_xs(0)
                nc.sync.dma_start(out=cos_sb, in_=cos_d.ap())
                nc.sync.dma_start(out=sin_sb, in_=sin_d.ap())
            nc.sync.dma_start(
                out=wv_sb, in_=wv_d.ap().rearrange("(kc p) f -> p kc f", p=128))
            ones_sb = consts.tile([128, 1], BF16)
            nc.vector.memset(ones_sb, 1.0)
            # warm the ACT exp table set early so phase 2's first exp
            # doesn't eat the ~2.7us ACT_TABLE_LOAD on the critical path
            warm = consts.tile([128, 1], F32)
            nc.vector.memset(warm, 0.0)
            nc.scalar.activation(warm, warm,
                                 mybir.ActivationFunctionType.Exp)

            qk_rot = qkp.tile([128, 8, T], BF16)   # m<4: q heads, m>=4: k heads
            v_sb = vp.tile([128, T // 128, 512], BF16)  # [t_part, t_tile, 4h*d]
            y_sb = yp.tile([128, HEADS_PER_CORE, T], BF16)  # [d, h, t]

            # ---- phase 1: qkv projection + RoPE ----
            for s in range(T // QS):
                ts = slice(s * QS, (s + 1) * QS)
                xs = xs_next
                if s + 1 < T // QS:
                    xs_next = load_xs(s + 1)
                for m in range(8):
                    # alternate pools: ps_s sits idle during phase 1, so
                    # qkv groups get an effective 5-deep PSUM rotation and
                    # the RoPE-read hold never stalls the next group
                    if cfg.get("p1_pool_mix", False) and m % 2 == 1:
                        ps = ps_s.tile([128, QS], F32, tag="ps_s")
                    else:
                        ps = ps_a.tile([128, QS], F32, tag="ps_a")
                    for kc in range(NKC):
                        nc.tensor.matmul(
                            ps, wqk_sb[:, kc, m * 128:(m + 1) * 128],
                            xs[:, kc, :],
                            start=(kc == 0), stop=(kc == NKC - 1))
                    # RoPE: rot = q * cos + swap_halves(q) * sin_signed.
                    # walrus rejects TensorTensor with two SBUF inputs at
                    # different base partitions, so the half swap is done by
                    # two SBUF->SBUF DMAs (partition-base offsets are legal
                    # for DMA); all DVE ops are then same-base.
                    if cfg["rope_from_sbuf"] == "act_split":
                        # bank freed after max(ACT straight copy, 2 short
                        # crossed DVE muls from PSUM) ~= 0.8us; the cos
                        # multiply then reads SBUF (same-base, legal)
                        q_sb = rope.tile([128, QS], F32, tag="q_sb")
                        nc.scalar.copy(q_sb, ps)
                        t2 = rope.tile([128, QS], F32, tag="t2")
                        nc.vector.tensor_mul(t2[0:64, :], ps[64:128, :],
                                             sin_sb[0:64, ts])
                        nc.vector.tensor_mul(t2[64:128, :], ps[0:64, :],
                                             sin_sb[64:128, ts])
                        t1 = rope.tile([128, QS], F32, tag="t1")
                        nc.vector.tensor_mul(t1, q_sb, cos_sb[:, ts])
                    elif cfg["rope_from_sbuf"]:
                        # single ACT copy frees the PSUM bank fast
                        q_sb = rope.tile([128, QS], F32, tag="q_sb")
                        nc.scalar.copy(q_sb, ps)
                        qs_sw = rope.tile([128, QS], F32, tag="qs_sw")
                        nc.sync.dma_start(out=qs_sw[0:64, :],
                                          in_=q_sb[64:128, :])
                        nc.sync.dma_start(out=qs_sw[64:128, :],
                                          in_=q_sb[0:64, :])
                        t1 = rope.tile([128, QS], F32, tag="t1")
                        nc.vector.tensor_mul(t1, q_sb, cos_sb[:, ts])
                        t2 = rope.tile([128, QS], F32, tag="t2")
                        nc.vector.tensor_mul(t2, qs_sw, sin_sb[:, ts])
                    else:
                        # PSUM+SBUF operands are exempt from the same-base
                        # rule: crossed reads come straight from PSUM
                        t1 = rope.tile([128, QS], F32, tag="t1")
                        nc.vector.tensor_mul(t1, ps, cos_sb[:, ts])
                        t2 = rope.tile([128, QS], F32, tag="t2")
                        nc.vector.tensor_mul(t2[0:64, :], ps[64:128, :],
                                             sin_sb[0:64, ts])
                        nc.vector.tensor_mul(t2[64:128, :], ps[0:64, :],
                                             sin_sb[64:128, ts])
                    nc.vector.tensor_add(qk_rot[:, m, ts], t1, t2)
                for i in range(QS // 128):
                    tt = 4 * s + i
                    if cfg.get("p1_pool_mix", False) and i % 2 == 1:
                        ps = ps_s.tile([128, 512], F32, tag="ps_s")
                    else:
                        ps = ps_a.tile([128, 512], F32, tag="ps_a")
                    for kc in range(NKC):
                        nc.tensor.matmul(
                            ps, xs[:, kc, i * 128:(i + 1) * 128],
                            wv_sb[:, kc, :],
                            start=(kc == 0), stop=(kc == NKC - 1))
                    last_strip = (s == T // QS - 1
                                  and cfg.get("v_tail_on_dve", True))
                    if cfg["v_copy_engine"] == "scalar" and not last_strip:
                        nc.scalar.copy(v_sb[:, tt, :], ps)
                    else:
                        # keep ACT free at the phase-1 tail so the first
                        # attention exp isn't queued behind these copies
                        nc.vector.tensor_copy(v_sb[:, tt, :], ps)

            # wp in its own slot lets the fused per-strip projection start
            # without waiting for the last wqk read; fall back to sharing
            # the wqk slot if SBUF is tight.
            if cfg["wp_own_slot"]:
                wp_sb = consts.tile([128, 4, E], BF16, tag="wp")
            else:
                wp_sb = wshare.tile([128, 4, E], BF16, tag="w")
            nc.sync.dma_start(
                out=wp_sb, in_=wp_d.ap().rearrange("(ec p) f -> p ec f", p=128))

            # ---- phase 2: attention ----
            from concourse import bass_isa

            def mask_diag(e_ap, qs_i, kt):
                # causal: keep where (tq + qs0) - (tk + kt0) >= 0
                nc.gpsimd.affine_select(
                    out=e_ap, in_=e_ap,
                    compare_op=mybir.AluOpType.is_ge,
                    fill=0.0,
                    base=qs_i * QS - kt * 128,
                    pattern=[[1, QS]],
                    channel_multiplier=-1)

            def proj_tile(ti):
                tsl = slice(ti * 128, (ti + 1) * 128)
                for fs in range(E // 512):
                    ps = ps_a.tile([128, 512], F32, tag="ps_a")
                    for h in range(HEADS_PER_CORE):
                        nc.tensor.matmul(
                            ps, y_sb[:, h, tsl],
                            wp_sb[:, h, fs * 512:(fs + 1) * 512],
                            start=(h == 0), stop=(h == 3))
                    ot = outp.tile([128, 512], F32, tag="ot")
                    nc.vector.tensor_copy(ot, ps)
                    nc.sync.dma_start(
                        out=out_d.ap()[tsl, fs * 512:(fs + 1) * 512], in_=ot)

            if cfg["fuse_proj"]:
                # strip 0 first (it only depends on phase-1 strip 0, so the
                # phase-1->2 transition is cheap), then longest-first so the
                # kernel tail is a short strip's attention + proj
                order = cfg.get("strip_order") or [1, 3, 2, 0]
                units = [(h, q) for q in order for h in range(HEADS_PER_CORE)]
            else:
                units = [(h, q) for h in range(HEADS_PER_CORE)
                         for q in range(T // QS)]
            for h, qs_i in units:
                if True:
                    qsl = slice(qs_i * QS, (qs_i + 1) * QS)
                    nk = 4 * qs_i + 4
                    psy = ps_y.tile([128, QS], F32, tag="ps_y")
                    psd = acc = None
                    if cfg["denom_on_pe"]:
                        psd = ps_d.tile([1, QS], F32, tag="ps_d")
                    else:
                        acc = denp.tile([128, QS], F32, tag="acc")

                    def consume_part(e_ap, kt, d0, w):
                        """denominator + attn@v for the live [128, w] slice
                        of one exp tile (columns d0..QS of the strip)."""
                        if cfg["denom_on_pe"]:
                            nc.tensor.matmul(psd[:, d0:d0 + w], ones_sb, e_ap,
                                             start=(kt == 0),
                                             stop=(kt == nk - 1),
                                             skip_group_check=True)
                        else:
                            if kt == 0:
                                nc.vector.tensor_copy(acc, e_ap)
                            else:
                                nc.vector.tensor_add(acc[:, d0:d0 + w],
                                                     acc[:, d0:d0 + w], e_ap)
                        nc.tensor.matmul(psy[:, d0:d0 + w],
                                         v_sb[:, kt, h * 128:(h + 1) * 128],
                                         e_ap, start=(kt == 0),
                                         stop=(kt == nk - 1),
                                         skip_group_check=True)

                    def consume(e_ap, kt):
                        consume_part(e_ap, kt, 0, QS)

                    if not cfg["exp_pair"]:
                        for kt in range(nk):
                            # diagonal tiles: columns tq < d0 are fully
                            # causal-masked, so shrink the score/exp/v work
                            # to the live N = QS - d0 columns
                            d0 = max(0, kt * 128 - qs_i * QS) \
                                if cfg.get("diag_shrink", True) else 0
                            w = QS - d0
                            pss = ps_s.tile([128, QS], F32, tag="ps_s")
                            nc.tensor.matmul(
                                pss[:, 0:w],
                                qk_rot[:, 4 + h, kt * 128:(kt + 1) * 128],
                                qk_rot[:, h,
                                       qs_i * QS + d0:(qs_i + 1) * QS],
                                start=True, stop=True)
                            e = expp.tile([128, QS], BF16, tag="e")
                            nc.scalar.activation(
                                e[:, 0:w], pss[:, 0:w],
                                mybir.ActivationFunctionType.Exp,
                                scale=SCALE)
                            if kt >= 4 * qs_i and w > 1:
                                # keep where local tq index j >= tk
                                nc.gpsimd.affine_select(
                                    out=e[:, 0:w], in_=e[:, 0:w],
                                    compare_op=mybir.AluOpType.is_ge,
                                    fill=0.0, base=0,
                                    pattern=[[1, w]],
                                    channel_multiplier=-1)
                            consume_part(e[:, 0:w], kt, d0, w)
                    else:
                        for kp in range(nk // 2):
                            pss = ps_s.tile([128, 2 * QS], F32, tag="ps_s")
                            for j in range(2):
                                kt = 2 * kp + j
                                nc.tensor.matmul(
                                    pss[:, j * QS:(j + 1) * QS],
                                    qk_rot[:, 4 + h, kt * 128:(kt + 1) * 128],
                                    qk_rot[:, h, qsl], start=True, stop=True)
                            e = expp.tile([128, 2 * QS], BF16, tag="e")
                            nc.scalar.activation(
                                e, pss, mybir.ActivationFunctionType.Exp,
                                scale=SCALE)
                            for j in range(2):
                                kt = 2 * kp + j
                                esl = e[:, j * QS:(j + 1) * QS]
                                if kt >= 4 * qs_i:
                                    mask_diag(esl, qs_i, kt)
                                consume(esl, kt)

                    # reciprocal + partition broadcast via DRAM round-trip
                    # (the gpsimd ucode broadcast is unsupported by this
                    # compiler build; DRAM reads may have partition step 0)
                    r = denp.tile([1, QS], F32, tag="r")
                    nc.vector.reciprocal(r, psd)
                    rdram = dramp.tile([1, QS], F32, tag="rd")
                    nc.sync.dma_start(out=rdram, in_=r)
                    rb = denp.tile([128, QS], F32, tag="rb")
                    rbc = bass.AP(tensor=rdram.tensor, offset=rdram.offset,
                                  ap=[[0, 128]] + list(rdram.ap[1:]))
                    nc.sync.dma_start(out=rb, in_=rbc)
                    if cfg["y_defer_scale"]:
                        # free the psy bank with one copy; the denominator
                        # scale lands later, off the PE critical path
                        nc.vector.tensor_copy(y_sb[:, h, qsl], psy)
                        nc.vector.tensor_mul(y_sb[:, h, qsl],
                                             y_sb[:, h, qsl], rb)
                    else:
                        nc.vector.tensor_mul(y_sb[:, h, qsl], psy, rb)
                if cfg["fuse_proj"] and h == HEADS_PER_CORE - 1:
                    for ti in range(4 * qs_i, 4 * qs_i + 4):
                        proj_tile(ti)

            # ---- phase 3: output projection (partial sums; host reduces) ----
            if not cfg["fuse_proj"]:
                for ti in range(T // 128):
                    proj_tile(ti)

    return nc


_HALF_PERM = np.concatenate([np.arange(0, 128, 2), np.arange(1, 128, 2)])


def make_in_maps(x, W_qkv, W_proj):
    """Host-side sharding: per-core input dict (bf16 where appropriate)."""
    x = np.asarray(x, dtype=np.float32)
    W_qkv = np.asarray(W_qkv, dtype=np.float32)
    W_proj = np.asarray(W_proj, dtype=np.float32)

    t = np.arange(T, dtype=np.float64)
    inv = 10000.0 ** (-np.arange(64, dtype=np.float64) / 64.0)
    ang = t[:, None] * inv[None, :]              # [T, 64]
    cos1 = np.cos(ang).astype(np.float32).T      # [64, T]
    sin1 = np.sin(ang).astype(np.float32).T
    cos_h = np.concatenate([cos1, cos1], axis=0)          # [128, T]
    sin_h = np.concatenate([-sin1, sin1], axis=0)         # [128, T]

    in_maps = []
    for c in range(N_CORES):
        b = c // HEADS_PER_CORE
        g = c % HEADS_PER_CORE
        h0 = g * HEADS_PER_CORE  # first head of this group
        xT = np.ascontiguousarray(x[b].T).astype(ml_dtypes.bfloat16)

        rows_q = []
        rows_k = []
        for h in range(h0, h0 + HEADS_PER_CORE):
            rows_q.append(W_qkv[h * 128 + _HALF_PERM, :])
            rows_k.append(W_qkv[E + h * 128 + _HALF_PERM, :])
        wqk = np.concatenate(rows_q + rows_k, axis=0)          # [1024, E]
        wqk = np.ascontiguousarray(wqk.T).astype(ml_dtypes.bfloat16)
        wv = W_qkv[2 * E + h0 * 128: 2 * E + h0 * 128 + 512, :]
        wv = np.ascontiguousarray(wv.T).astype(ml_dtypes.bfloat16)
        wp = W_proj[:, h0 * 128: h0 * 128 + 512]
        wp = np.ascontiguousarray(wp.T).astype(ml_dtypes.bfloat16)

        in_maps.append({
            "xT": xT, "wqk": wqk, "wv": wv, "wp": wp,
            "cos": cos_h, "sin": sin_h,
        })
    return in_maps


_RUNNER = None


def _get_runner():
    """Build the Bass program once and return a cached PJRT runner."""
    global _RUNNER
    if _RUNNER is not None:
        return _RUNNER

    import jax
    import jax.numpy as jnp  # noqa: F401
    from jax.sharding import Mesh, PartitionSpec
    from jax.experimental.shard_map import shard_map
    from concourse import bass2jax

    nc = build_program()
    bass2jax.install_neuronx_cc_hook()

    partition_name = (nc.partition_id_tensor.name
                      if nc.partition_id_tensor else None)
    in_names, out_names, out_avals = [], [], []
    for alloc in nc.m.functions[0].allocations:
        if not isinstance(alloc, mybir.MemoryLocationSet):
            continue
        name = alloc.memorylocations[0].name
        if alloc.kind == "ExternalInput":
            if name != partition_name:
                in_names.append(name)
        elif alloc.kind == "ExternalOutput":
            out_names.append(name)
            out_avals.append(jax.core.ShapedArray(
                tuple(alloc.tensor_shape), mybir.dt.np(alloc.dtype)))
    n_params = len(in_names)
    n_outs = len(out_names)
    zero_shapes = [(a.shape, a.dtype) for a in out_avals]
    all_in_names = list(in_names) + list(out_names)
    if partition_name is not None:
        all_in_names.append(partition_name)

    def _body(*args):
        operands = list(args)
        if partition_name is not None:
            operands.append(bass2jax.partition_id_tensor())
        outs = bass2jax._bass_exec_p.bind(
            *operands,
            out_avals=tuple(out_avals),
            in_names=tuple(all_in_names),
            out_names=tuple(out_names),
            lowering_input_output_aliases=(),
            sim_require_finite=True,
            sim_require_nnan=True,
            nc=nc,
        )
        return tuple(outs)

    devices = jax.devices()[:N_CORES]
    mesh = Mesh(np.asarray(devices), ("core",))
    donate = tuple(range(n_params, n_params + n_outs))
    sharded = jax.jit(
        shard_map(_body, mesh=mesh,
                  in_specs=(PartitionSpec("core"),) * (n_params + n_outs),
                  out_specs=(PartitionSpec("core"),) * n_outs,
                  check_rep=False),
        donate_argnums=donate, keep_unused=True)

    class Runner:
        def __init__(self):
            self.sharded = sharded
            self.mesh = mesh
            self.in_names = in_names
            self.out_names = out_names
            self.zero_shapes = zero_shapes

        def prep_inputs(self, in_maps):
            return [
                np.concatenate(
                    [np.asarray(in_maps[c][nm]) for c in range(N_CORES)], axis=0)
                for nm in in_names
            ]

        def zero_set(self):
            return [
                np.zeros((N_CORES * s[0], *s[1:]), d) for (s, d) in zero_shapes
            ]

        def call(self, concat_in, concat_zeros):
            return sharded(*concat_in, *concat_zeros)

        def run(self, in_maps):
            out_arrs = self.call(self.prep_inputs(in_maps), self.zero_set())
            for o in out_arrs:
                o.block_until_ready()
            return [
                {nm: np.asarray(out_arrs[i]).reshape(
                    N_CORES, *zero_shapes[i][0])[c]
                 for i, nm in enumerate(out_names)}
                for c in range(N_CORES)
            ]

    _RUNNER = Runner()
    return _RUNNER


def kernel(x, W_qkv, W_proj):
    in_maps = make_in_maps(x, W_qkv, W_proj)
    runner = _get_runner()
    results = runner.run(in_maps)
    out = np.zeros((B, T, E), dtype=np.float32)
    for c in range(N_CORES):
        out[c // HEADS_PER_CORE] += results[c]["out"]
    return out



# revision 23
# speedup vs baseline: 243.8850x; 243.8850x over previous
"""Causal self-attention (B=2, T=2048, E=2048, H=16, D=128) on 8 TRN2 cores.

Sharding: core c handles batch b = c//4 and head group g = c%4 (4 heads).
 - W_qkv is split column-wise (per head group) -> each core projects only its
   heads' q/k/v. q,k are produced in [d, t] layout, v in [t, d] layout.
 - RoPE is applied on-device in a half-split basis (host permutes W rows for
   q/k so that rotation pairs are (i, i+64) instead of (2i, 2i+1); scores are
   invariant because q and k get the same permutation).
 - scores are computed transposed (S^T tiles [t_k=128, t_q=512]); softmax is
   max-free (scores*scale stay in [-10, 10] for this problem) with the
   denominator accumulated by a ones-matmul on the PE, and the 1/denom scale
   applied to the attention output before the output projection.
 - W_proj is split row-wise; each core emits a partial [T, E] output and the
   host sums the 4 partials per batch.

All matmuls run in bf16 (fp32 PSUM accumulation). Measured end-to-end
absmax-relative error vs the fp32 reference is ~4e-3.
"""

import math
import os

import numpy as np
import ml_dtypes

import concourse.bass as bass
import concourse.mybir as mybir
import concourse.tile as tile
from concourse.vector_clock import ScopedClock

BF16 = mybir.dt.bfloat16
F32 = mybir.dt.float32

B, T, E = 2, 2048, 2048
H, D = 16, 128
N_CORES = 8
HEADS_PER_CORE = 4
KT = 128          # t_k tile (partitions of score tiles)
QS = 512          # t_q strip (free dim of score tiles)
NKC = E // 128    # contraction chunks for the projections
SCALE = 1.0 / math.sqrt(D)

# The walrus build in this container encodes only one sync-wait command per
# instruction (a 3-wait Drain and a 3-wait TensorTensor both fail codegen
# with "Too many sync wait commands"). Tile's scheduler emits multi-wait
# instructions freely, so after scheduling we move each excess wait onto its
# own preceding same-engine NoOp.
_MAX_WAITS = 1


def _split_excess_waits(nc, max_waits=_MAX_WAITS):
    f = nc.m.functions[0]

    def overloaded(ins):
        si = ins.sync_info
        return si is not None and len(si.on_wait) > max_waits

    plan = {}  # bb name -> set of overloaded instruction names
    for bb in f.blocks:
        names = {ins.name for ins in bb.instructions if overloaded(ins)}
        if names:
            plan[bb.name] = names
    if not plan:
        return

    nop_map = {}   # overloaded inst name -> [nop instruction, ...]
    nop_names = set()
    for bb in f.blocks:
        todo = plan.get(bb.name)
        if not todo:
            continue
        for ins in bb.instructions:
            if ins.name not in todo:
                continue
            si = ins.sync_info
            waits = list(si.on_wait)
            excess, keep = waits[:-max_waits], waits[-max_waits:]
            nops = []
            for w in excess:
                nop = nc.engines[ins.engine].nop(nofuse=True).ins
                nop.sync_info = mybir.SyncInfo(on_wait=[w], on_update=[])
                nops.append(nop)
                nop_names.add(nop.name)
            ins.sync_info = mybir.SyncInfo(
                on_wait=keep, on_update=list(si.on_update))
            nop_map[ins.name] = nops

    # The nop builder appended the new instructions to the tail block;
    # remove them from wherever they landed, then splice each in front of
    # its target instruction (same engine => preserves engine order).
    for bb in f.blocks:
        lst = [i for i in bb.instructions if i.name not in nop_names]
        if bb.name in plan:
            out = []
            for i in lst:
                out.extend(nop_map.get(i.name, ()))
                out.append(i)
            lst = out
        bb.instructions = lst


class _TileContext(tile.TileContext):
    def _drain_and_barrier(self, tick_clock, wait_clock):
        nc = self.nc
        drain_inst = nc.sync.drain()
        wait_clock.add_sem_waits(
            drain_inst.ins, ScopedClock({None: tick_clock.global_clock})
        )
        nc.all_engine_barrier()
        assert self.sems is not None
        popped = nc._tile_sem_poison_stack.pop()
        assert popped is self._sem_poison
        nc.clear_and_free_semaphores(list(self.sems.allocated().values()))
        nc.all_engine_barrier()
        _split_excess_waits(nc)


DEFAULT_CFG = dict(
    ps_a=2, ps_s=3, ps_y=2, ps_d=1,
    expp=6, rope=2, xsp=2, outp=4, denp=3,
    denom_acc="pe",     # 'pe': per-tile ones-matmul; 'pool'/'dve': elementwise
                        # accumulate on that engine + one final PE reduce
                        # (measured: elementwise engines are ~10x slower per
                        # column than PE bf16 streaming -- 'pe' wins)
    acc_dt="bf16",      # accumulator dtype for denom_acc pool/dve
    v_copy_engine="scalar",
    exp_pair=False,     # one ACT exp over two score tiles (2-bank PSUM)
    rope_from_sbuf=False,  # crossed RoPE reads direct from PSUM (walrus-legal)
    split_dmas=True,      # chunk weight/x DMAs so first matmuls start early
    split_cs=True,        # cos/sin DMA per strip: strip 0 early, rest late
    cs_bf16=True,         # cos/sin tables in bf16 (frees 8KB/partition)
    y_defer_scale=True,   # copy psy out unscaled; apply 1/denom in place
    tail_scale_pool=False,  # last strip's y copy/scale on Pool (slower: off)
    out_copy_engine="vector",  # PSUM->SBUF copy engine for proj output
    fuse_proj=True,       # emit the proj for each q-strip right after it
    wp_own_slot=True,     # load W_proj into its own tile at kernel start
    hw_loop=1,            # >1: wrap the body in a device-side For_i loop
)


def build_program(cfg=None, n_iters=1):
    cfg = {**DEFAULT_CFG, **(cfg or {})}
    nc = bass.Bass("TRN2", target_bir_lowering=False, debug=False,
                   num_devices=N_CORES)

    xT_d = nc.dram_tensor("xT", [E, T], BF16, kind="ExternalInput")
    # wqk arrives host-permuted as [p, m, kc, f] (p-major) so each m-chunk
    # DMA moves 4KB-contiguous runs per partition instead of 256B ones
    wqk_d = nc.dram_tensor("wqk", [128, 8 * NKC * 128], BF16,
                           kind="ExternalInput")
    wv_d = nc.dram_tensor("wv", [E, 512], BF16, kind="ExternalInput")
    wp_d = nc.dram_tensor("wp", [512, E], BF16, kind="ExternalInput")
    CS = BF16 if cfg["cs_bf16"] else F32
    cos_d = nc.dram_tensor("cos", [128, T], CS, kind="ExternalInput")
    sin_d = nc.dram_tensor("sin", [128, T], CS, kind="ExternalInput")
    out_d = nc.dram_tensor("out", [T, E], F32, kind="ExternalOutput")

    from contextlib import ExitStack

    with _TileContext(nc) as tc, ExitStack() as ctx:
        consts = ctx.enter_context(tc.tile_pool(name="consts", bufs=1))
        wshare = ctx.enter_context(tc.tile_pool(name="wshare", bufs=1))
        xsp = ctx.enter_context(tc.tile_pool(name="xsp", bufs=cfg["xsp"]))
        qkp = ctx.enter_context(tc.tile_pool(name="qkp", bufs=1))
        vp = ctx.enter_context(tc.tile_pool(name="vp", bufs=1))
        yp = ctx.enter_context(tc.tile_pool(name="yp", bufs=1))
        rope = ctx.enter_context(tc.tile_pool(name="rope", bufs=cfg["rope"]))
        expp = ctx.enter_context(tc.tile_pool(name="expp", bufs=cfg["expp"]))
        denp = ctx.enter_context(tc.tile_pool(name="denp", bufs=cfg.get("denp", 2)))
        outp = ctx.enter_context(tc.tile_pool(name="outp", bufs=cfg["outp"]))
        ps_a = ctx.enter_context(
            tc.tile_pool(name="ps_a", bufs=cfg["ps_a"], space="PSUM"))
        ps_s = ctx.enter_context(
            tc.tile_pool(name="ps_s", bufs=cfg["ps_s"], space="PSUM"))
        ps_y = ctx.enter_context(
            tc.tile_pool(name="ps_y", bufs=cfg["ps_y"], space="PSUM"))
        ps_d = ctx.enter_context(
            tc.tile_pool(name="ps_d", bufs=cfg["ps_d"], space="PSUM"))
        accp = None
        if cfg["denom_acc"] != "pe":
            accp = ctx.enter_context(
                tc.tile_pool(name="accp", bufs=cfg.get("accp", 2)))

        dramp = ctx.enter_context(
            tc.tile_pool(name="dramp", bufs=cfg.get("dramp", 4), space="DRAM"))
        if cfg["hw_loop"] > 1:
            ctx.enter_context(tc.For_i(0, cfg["hw_loop"]))
        for _it in range(n_iters):
            # ---- constants / weights ----
            wqk_re = wqk_d.ap().rearrange("p (m kc f) -> p m kc f",
                                          m=8, kc=NKC)
            wqk_sb = wshare.tile([128, 8, NKC, 128], BF16, tag="w")
            cos_sb = consts.tile([128, T], CS)
            sin_sb = consts.tile([128, T], CS)
            wv_sb = consts.tile([128, NKC, 512], BF16)

            def load_xs(s):
                xs = xsp.tile([128, NKC, QS], BF16, tag="xs")
                src = xT_d.ap()[:, s * QS:(s + 1) * QS].rearrange(
                    "(kc p) t -> p kc t", p=128)
                if cfg["split_dmas"]:
                    for j in range(4):
                        nc.sync.dma_start(out=xs[:, 4 * j:4 * j + 4, :],
                                          in_=src[:, 4 * j:4 * j + 4, :])
                else:
                    nc.sync.dma_start(out=xs, in_=src)
                return xs

            if cfg["split_dmas"]:
                # loads in exact consumption order: the first M-tile group
                # consumes (wqk[:, kc, 0:128], xs[:, kc, :]) for kc = 0..15,
                # so interleave 4-kc chunks of both streams
                xs_next = xsp.tile([128, NKC, QS], BF16, tag="xs")
                xs0_src = xT_d.ap()[:, 0:QS].rearrange(
                    "(kc p) t -> p kc t", p=128)
                xq = nc.gpsimd if cfg.get("xs0_gpsimd", False) else nc.sync
                for j in range(4):
                    kcs = slice(4 * j, 4 * j + 4)
                    nc.sync.dma_start(out=wqk_sb[:, 0, kcs, :],
                                      in_=wqk_re[:, 0, kcs, :])
                    xq.dma_start(out=xs_next[:, kcs, :],
                                 in_=xs0_src[:, kcs, :])
                n_early = cfg.get("wqk_early", 1)
                for m in range(1, 1 + n_early):
                    nc.sync.dma_start(out=wqk_sb[:, m], in_=wqk_re[:, m])
                if cfg["split_cs"]:
                    # RoPE needs only strip 0's cos/sin early; the rest can
                    # queue behind the remaining wqk m-tiles
                    nc.sync.dma_start(out=cos_sb[:, 0:QS],
                                      in_=cos_d.ap()[:, 0:QS])
                    nc.sync.dma_start(out=sin_sb[:, 0:QS],
                                      in_=sin_d.ap()[:, 0:QS])
                else:
                    nc.sync.dma_start(out=cos_sb, in_=cos_d.ap())
                    nc.sync.dma_start(out=sin_sb, in_=sin_d.ap())
                for m in range(1 + n_early, 8):
                    nc.sync.dma_start(out=wqk_sb[:, m], in_=wqk_re[:, m])
                if cfg["split_cs"]:
                    nc.sync.dma_start(out=cos_sb[:, QS:T],
                                      in_=cos_d.ap()[:, QS:T])
                    nc.sync.dma_start(out=sin_sb[:, QS:T],
                                      in_=sin_d.ap()[:, QS:T])
            else:
                nc.sync.dma_start(out=wqk_sb, in_=wqk_re)
                xs_next = load_xs(0)
                nc.sync.dma_start(out=cos_sb, in_=cos_d.ap())
                nc.sync.dma_start(out=sin_sb, in_=sin_d.ap())
            nc.sync.dma_start(
                out=wv_sb, in_=wv_d.ap().rearrange("(kc p) f -> p kc f", p=128))
            ones_dt = BF16 if (cfg["denom_acc"] == "pe"
                              or cfg["acc_dt"] == "bf16") else F32
            ones_sb = consts.tile([128, 1], ones_dt)
            nc.vector.memset(ones_sb, 1.0)
            # warm the ACT exp table set early so phase 2's first exp
            # doesn't eat the ~2.7us ACT_TABLE_LOAD on the critical path
            warm = consts.tile([128, 1], F32)
            nc.vector.memset(warm, 0.0)
            nc.scalar.activation(warm, warm,
                                 mybir.ActivationFunctionType.Exp)

            qk_rot = qkp.tile([128, 8, T], BF16)   # m<4: q heads, m>=4: k heads
            v_sb = vp.tile([128, T // 128, 512], BF16)  # [t_part, t_tile, 4h*d]
            y_sb = yp.tile([128, HEADS_PER_CORE, T], BF16)  # [d, h, t]

            # ---- phase 1: qkv projection + RoPE ----
            for s in range(T // QS):
                ts = slice(s * QS, (s + 1) * QS)
                xs = xs_next
                if s + 1 < T // QS:
                    xs_next = load_xs(s + 1)
                for m in range(8):
                    # alternate pools: ps_s sits idle during phase 1, so
                    # qkv groups get an effective 5-deep PSUM rotation and
                    # the RoPE-read hold never stalls the next group
                    if cfg.get("p1_pool_mix", False) and m % 2 == 1:
                        ps = ps_s.tile([128, QS], F32, tag="ps_s")
                    else:
                        ps = ps_a.tile([128, QS], F32, tag="ps_a")
                    for kc in range(NKC):
                        nc.tensor.matmul(
                            ps, wqk_sb[:, m, kc, :], xs[:, kc, :],
                            start=(kc == 0), stop=(kc == NKC - 1))
                    # RoPE: rot = q * cos + swap_halves(q) * sin_signed.
                    # walrus rejects TensorTensor with two SBUF inputs at
                    # different base partitions, so the half swap is done by
                    # two SBUF->SBUF DMAs (partition-base offsets are legal
                    # for DMA); all DVE ops are then same-base.
                    if cfg["rope_from_sbuf"] == "act_split":
                        # bank freed after max(ACT straight copy, 2 short
                        # crossed DVE muls from PSUM) ~= 0.8us; the cos
                        # multiply then reads SBUF (same-base, legal)
                        q_sb = rope.tile([128, QS], F32, tag="q_sb")
                        nc.scalar.copy(q_sb, ps)
                        t2 = rope.tile([128, QS], F32, tag="t2")
                        nc.vector.tensor_mul(t2[0:64, :], ps[64:128, :],
                                             sin_sb[0:64, ts])
                        nc.vector.tensor_mul(t2[64:128, :], ps[0:64, :],
                                             sin_sb[64:128, ts])
                        t1 = rope.tile([128, QS], F32, tag="t1")
                        nc.vector.tensor_mul(t1, q_sb, cos_sb[:, ts])
                    elif cfg["rope_from_sbuf"]:
                        # single ACT copy frees the PSUM bank fast
                        q_sb = rope.tile([128, QS], F32, tag="q_sb")
                        nc.scalar.copy(q_sb, ps)
                        qs_sw = rope.tile([128, QS], F32, tag="qs_sw")
                        nc.sync.dma_start(out=qs_sw[0:64, :],
                                          in_=q_sb[64:128, :])
                        nc.sync.dma_start(out=qs_sw[64:128, :],
                                          in_=q_sb[0:64, :])
                        t1 = rope.tile([128, QS], F32, tag="t1")
                        nc.vector.tensor_mul(t1, q_sb, cos_sb[:, ts])
                        t2 = rope.tile([128, QS], F32, tag="t2")
                        nc.vector.tensor_mul(t2, qs_sw, sin_sb[:, ts])
                    else:
                        # PSUM+SBUF operands are exempt from the same-base
                        # rule: crossed reads come straight from PSUM
                        t1 = rope.tile([128, QS], F32, tag="t1")
                        nc.vector.tensor_mul(t1, ps, cos_sb[:, ts])
                        t2 = rope.tile([128, QS], F32, tag="t2")
                        nc.vector.tensor_mul(t2[0:64, :], ps[64:128, :],
                                             sin_sb[0:64, ts])
                        nc.vector.tensor_mul(t2[64:128, :], ps[0:64, :],
                                             sin_sb[64:128, ts])
                    nc.vector.tensor_add(qk_rot[:, m, ts], t1, t2)
                for i in range(QS // 128):
                    tt = 4 * s + i
                    if cfg.get("p1_pool_mix", False) and i % 2 == 1:
                        ps = ps_s.tile([128, 512], F32, tag="ps_s")
                    else:
                        ps = ps_a.tile([128, 512], F32, tag="ps_a")
                    for kc in range(NKC):
                        nc.tensor.matmul(
                            ps, xs[:, kc, i * 128:(i + 1) * 128],
                            wv_sb[:, kc, :],
                            start=(kc == 0), stop=(kc == NKC - 1))
                    last_strip = (s == T // QS - 1
                                  and cfg.get("v_tail_on_dve", True))
                    if cfg["v_copy_engine"] == "scalar" and not last_strip:
                        nc.scalar.copy(v_sb[:, tt, :], ps)
                    else:
                        # keep ACT free at the phase-1 tail so the first
                        # attention exp isn't queued behind these copies
                        nc.vector.tensor_copy(v_sb[:, tt, :], ps)

            # wp in its own slot lets the fused per-strip projection start
            # without waiting for the last wqk read; fall back to sharing
            # the wqk slot if SBUF is tight.
            if cfg["wp_own_slot"]:
                wp_sb = consts.tile([128, 4, E], BF16, tag="wp")
            else:
                wp_sb = wshare.tile([128, 4, E], BF16, tag="w")
            nc.sync.dma_start(
                out=wp_sb, in_=wp_d.ap().rearrange("(ec p) f -> p ec f", p=128))

            # ---- phase 2: attention ----
            from concourse import bass_isa

            def mask_diag(e_ap, qs_i, kt):
                # causal: keep where (tq + qs0) - (tk + kt0) >= 0
                nc.gpsimd.affine_select(
                    out=e_ap, in_=e_ap,
                    compare_op=mybir.AluOpType.is_ge,
                    fill=0.0,
                    base=qs_i * QS - kt * 128,
                    pattern=[[1, QS]],
                    channel_multiplier=-1)

            def proj_tile(ti):
                tsl = slice(ti * 128, (ti + 1) * 128)
                for fs in range(E // 512):
                    ps = ps_a.tile([128, 512], F32, tag="ps_a")
                    for h in range(HEADS_PER_CORE):
                        nc.tensor.matmul(
                            ps, y_sb[:, h, tsl],
                            wp_sb[:, h, fs * 512:(fs + 1) * 512],
                            start=(h == 0), stop=(h == 3))
                    ot = outp.tile([128, 512], F32, tag="ot")
                    oce = cfg["out_copy_engine"]
                    if oce == "alt":
                        oce = "scalar" if fs % 2 else "vector"
                    if oce == "scalar":
                        nc.scalar.copy(ot, ps)
                    elif oce == "pool":
                        nc.gpsimd.tensor_copy(ot, ps)
                    else:
                        nc.vector.tensor_copy(ot, ps)
                    nc.sync.dma_start(
                        out=out_d.ap()[tsl, fs * 512:(fs + 1) * 512], in_=ot)

            if cfg["fuse_proj"]:
                # strip 0 first (it only depends on phase-1 strip 0, so the
                # phase-1->2 transition is cheap), then longest-first so the
                # kernel tail is a short strip's attention + proj
                order = cfg.get("strip_order") or [1, 3, 2, 0]
                units = [(h, q) for q in order for h in range(HEADS_PER_CORE)]
                last_q = order[-1]
            else:
                units = [(h, q) for h in range(HEADS_PER_CORE)
                         for q in range(T // QS)]
                last_q = T // QS - 1
            for h, qs_i in units:
                if True:
                    qsl = slice(qs_i * QS, (qs_i + 1) * QS)
                    nk = 4 * qs_i + 4
                    psy = ps_y.tile([128, QS], F32, tag="ps_y")
                    psd = acc = None
                    acc_dt = F32 if cfg["acc_dt"] == "f32" else BF16
                    if cfg["denom_acc"] == "pe":
                        psd = ps_d.tile([1, QS], F32, tag="ps_d")
                    else:
                        acc = accp.tile([128, QS], acc_dt, tag="acc")
                    acc_eng = nc.gpsimd if cfg["denom_acc"] == "pool" \
                        else nc.vector

                    def consume_part(e_ap, kt, d0, w):
                        """denominator + attn@v for the live [128, w] slice
                        of one exp tile (columns d0..QS of the strip)."""
                        if cfg["denom_acc"] == "pe":
                            nc.tensor.matmul(psd[:, d0:d0 + w], ones_sb, e_ap,
                                             start=(kt == 0),
                                             stop=(kt == nk - 1),
                                             skip_group_check=True)
                        else:
                            if kt == 0:
                                acc_eng.tensor_copy(acc, e_ap)
                            else:
                                acc_eng.tensor_add(acc[:, d0:d0 + w],
                                                   acc[:, d0:d0 + w], e_ap)
                        nc.tensor.matmul(psy[:, d0:d0 + w],
                                         v_sb[:, kt, h * 128:(h + 1) * 128],
                                         e_ap, start=(kt == 0),
                                         stop=(kt == nk - 1),
                                         skip_group_check=True)

                    def consume(e_ap, kt):
                        consume_part(e_ap, kt, 0, QS)

                    if not cfg["exp_pair"]:
                        for kt in range(nk):
                            # diagonal tiles: columns tq < d0 are fully
                            # causal-masked, so shrink the score/exp/v work
                            # to the live N = QS - d0 columns
                            d0 = max(0, kt * 128 - qs_i * QS) \
                                if cfg.get("diag_shrink", True) else 0
                            w = QS - d0
                            pss = ps_s.tile([128, QS], F32, tag="ps_s")
                            nc.tensor.matmul(
                                pss[:, 0:w],
                                qk_rot[:, 4 + h, kt * 128:(kt + 1) * 128],
                                qk_rot[:, h,
                                       qs_i * QS + d0:(qs_i + 1) * QS],
                                start=True, stop=True)
                            e = expp.tile([128, QS], BF16, tag="e")
                            nc.scalar.activation(
                                e[:, 0:w], pss[:, 0:w],
                                mybir.ActivationFunctionType.Exp,
                                scale=SCALE)
                            if kt >= 4 * qs_i and w > 1:
                                # keep where local tq index j >= tk
                                nc.gpsimd.affine_select(
                                    out=e[:, 0:w], in_=e[:, 0:w],
                                    compare_op=mybir.AluOpType.is_ge,
                                    fill=0.0, base=0,
                                    pattern=[[1, w]],
                                    channel_multiplier=-1)
                            consume_part(e[:, 0:w], kt, d0, w)
                    else:
                        for kp in range(nk // 2):
                            pss = ps_s.tile([128, 2 * QS], F32, tag="ps_s")
                            for j in range(2):
                                kt = 2 * kp + j
                                nc.tensor.matmul(
                                    pss[:, j * QS:(j + 1) * QS],
                                    qk_rot[:, 4 + h, kt * 128:(kt + 1) * 128],
                                    qk_rot[:, h, qsl], start=True, stop=True)
                            e = expp.tile([128, 2 * QS], BF16, tag="e")
                            nc.scalar.activation(
                                e, pss, mybir.ActivationFunctionType.Exp,
                                scale=SCALE)
                            for j in range(2):
                                kt = 2 * kp + j
                                esl = e[:, j * QS:(j + 1) * QS]
                                if kt >= 4 * qs_i:
                                    mask_diag(esl, qs_i, kt)
                                consume(esl, kt)

                    if cfg["denom_acc"] != "pe":
                        # single partition-reduce of the elementwise
                        # accumulator: psd[0, q] = sum_p acc[p, q]
                        psd = ps_d.tile([1, QS], F32, tag="ps_d")
                        nc.tensor.matmul(psd, ones_sb, acc,
                                         start=True, stop=True)
                    # reciprocal + partition broadcast via DRAM round-trip
                    # (the gpsimd ucode broadcast is unsupported by this
                    # compiler build; DRAM reads may have partition step 0)
                    r = denp.tile([1, QS], F32, tag="r")
                    nc.vector.reciprocal(r, psd)
                    rdram = dramp.tile([1, QS], F32, tag="rd")
                    nc.sync.dma_start(out=rdram, in_=r)
                    rb = denp.tile([128, QS], F32, tag="rb")
                    rbc = bass.AP(tensor=rdram.tensor, offset=rdram.offset,
                                  ap=[[0, 128]] + list(rdram.ap[1:]))
                    nc.sync.dma_start(out=rb, in_=rbc)
                    y_eng = nc.vector
                    if cfg["tail_scale_pool"] and cfg["fuse_proj"] \
                            and qs_i == last_q:
                        # at the kernel tail DVE is the last-proj critical
                        # path; Pool is idle there
                        y_eng = nc.gpsimd
                    if cfg["y_defer_scale"]:
                        # free the psy bank with one copy; the denominator
                        # scale lands later, off the PE critical path
                        y_eng.tensor_copy(y_sb[:, h, qsl], psy)
                        y_eng.tensor_mul(y_sb[:, h, qsl],
                                         y_sb[:, h, qsl], rb)
                    else:
                        y_eng.tensor_mul(y_sb[:, h, qsl], psy, rb)
                if cfg["fuse_proj"] and h == HEADS_PER_CORE - 1:
                    for ti in range(4 * qs_i, 4 * qs_i + 4):
                        proj_tile(ti)

            # ---- phase 3: output projection (partial sums; host reduces) ----
            if not cfg["fuse_proj"]:
                for ti in range(T // 128):
                    proj_tile(ti)

    return nc


_HALF_PERM = np.concatenate([np.arange(0, 128, 2), np.arange(1, 128, 2)])


def make_in_maps(x, W_qkv, W_proj):
    """Host-side sharding: per-core input dict (bf16 where appropriate)."""
    x = np.asarray(x, dtype=np.float32)
    W_qkv = np.asarray(W_qkv, dtype=np.float32)
    W_proj = np.asarray(W_proj, dtype=np.float32)

    t = np.arange(T, dtype=np.float64)
    inv = 10000.0 ** (-np.arange(64, dtype=np.float64) / 64.0)
    ang = t[:, None] * inv[None, :]              # [T, 64]
    cos1 = np.cos(ang).astype(np.float32).T      # [64, T]
    sin1 = np.sin(ang).astype(np.float32).T
    cos_h = np.concatenate([cos1, cos1], axis=0)          # [128, T]
    sin_h = np.concatenate([-sin1, sin1], axis=0)         # [128, T]
    if DEFAULT_CFG["cs_bf16"]:
        cos_h = cos_h.astype(ml_dtypes.bfloat16)
        sin_h = sin_h.astype(ml_dtypes.bfloat16)

    in_maps = []
    for c in range(N_CORES):
        b = c // HEADS_PER_CORE
        g = c % HEADS_PER_CORE
        h0 = g * HEADS_PER_CORE  # first head of this group
        xT = np.ascontiguousarray(x[b].T).astype(ml_dtypes.bfloat16)

        rows_q = []
        rows_k = []
        for h in range(h0, h0 + HEADS_PER_CORE):
            rows_q.append(W_qkv[h * 128 + _HALF_PERM, :])
            rows_k.append(W_qkv[E + h * 128 + _HALF_PERM, :])
        wqk = np.concatenate(rows_q + rows_k, axis=0)          # [1024, E]
        # device layout [p, m, kc, f]: every per-partition run of an m-chunk
        # (and of the interleaved m=0 kc-chunks) is contiguous in DRAM
        wqk = np.ascontiguousarray(
            wqk.reshape(8, 128, NKC, 128).transpose(3, 0, 2, 1)
            .reshape(128, 8 * NKC * 128)).astype(ml_dtypes.bfloat16)
        wv = W_qkv[2 * E + h0 * 128: 2 * E + h0 * 128 + 512, :]
        wv = np.ascontiguousarray(wv.T).astype(ml_dtypes.bfloat16)
        wp = W_proj[:, h0 * 128: h0 * 128 + 512]
        wp = np.ascontiguousarray(wp.T).astype(ml_dtypes.bfloat16)

        in_maps.append({
            "xT": xT, "wqk": wqk, "wv": wv, "wp": wp,
            "cos": cos_h, "sin": sin_h,
        })
    return in_maps


_RUNNERS = {}


def _get_runner(cfg=None, _key=None):
    """Build the Bass program once per cfg and return a cached PJRT runner."""
    key = _key if _key is not None else (
        None if cfg is None else tuple(sorted(cfg.items())))
    if key in _RUNNERS:
        return _RUNNERS[key]

    import jax
    import jax.numpy as jnp  # noqa: F401
    from jax.sharding import Mesh, PartitionSpec
    from jax.experimental.shard_map import shard_map
    from concourse import bass2jax

    nc = build_program(cfg)
    bass2jax.install_neuronx_cc_hook()

    partition_name = (nc.partition_id_tensor.name
                      if nc.partition_id_tensor else None)
    in_names, out_names, out_avals = [], [], []
    for alloc in nc.m.functions[0].allocations:
        if not isinstance(alloc, mybir.MemoryLocationSet):
            continue
        name = alloc.memorylocations[0].name
        if alloc.kind == "ExternalInput":
            if name != partition_name:
                in_names.append(name)
        elif alloc.kind == "ExternalOutput":
            out_names.append(name)
            out_avals.append(jax.core.ShapedArray(
                tuple(alloc.tensor_shape), mybir.dt.np(alloc.dtype)))
    n_params = len(in_names)
    n_outs = len(out_names)
    zero_shapes = [(a.shape, a.dtype) for a in out_avals]
    all_in_names = list(in_names) + list(out_names)
    if partition_name is not None:
        all_in_names.append(partition_name)

    def _body(*args):
        operands = list(args)
        if partition_name is not None:
            operands.append(bass2jax.partition_id_tensor())
        outs = bass2jax._bass_exec_p.bind(
            *operands,
            out_avals=tuple(out_avals),
            in_names=tuple(all_in_names),
            out_names=tuple(out_names),
            lowering_input_output_aliases=(),
            sim_require_finite=True,
            sim_require_nnan=True,
            nc=nc,
        )
        return tuple(outs)

    devices = jax.devices()[:N_CORES]
    mesh = Mesh(np.asarray(devices), ("core",))
    donate = tuple(range(n_params, n_params + n_outs))
    sharded = jax.jit(
        shard_map(_body, mesh=mesh,
                  in_specs=(PartitionSpec("core"),) * (n_params + n_outs),
                  out_specs=(PartitionSpec("core"),) * n_outs,
                  check_rep=False),
        donate_argnums=donate, keep_unused=True)

    class Runner:
        def __init__(self):
            self.sharded = sharded
            self.mesh = mesh
            self.in_names = in_names
            self.out_names = out_names
            self.zero_shapes = zero_shapes

        def prep_inputs(self, in_maps):
            return [
                np.concatenate(
                    [np.asarray(in_maps[c][nm]) for c in range(N_CORES)], axis=0)
                for nm in in_names
            ]

        def zero_set(self):
            return [
                np.zeros((N_CORES * s[0], *s[1:]), d) for (s, d) in zero_shapes
            ]

        def call(self, concat_in, concat_zeros):
            return sharded(*concat_in, *concat_zeros)

        def run(self, in_maps):
            out_arrs = self.call(self.prep_inputs(in_maps), self.zero_set())
            for o in out_arrs:
                o.block_until_ready()
            return [
                {nm: np.asarray(out_arrs[i]).reshape(
                    N_CORES, *zero_shapes[i][0])[c]
                 for i, nm in enumerate(out_names)}
                for c in range(N_CORES)
            ]

    _RUNNERS[key] = Runner()
    return _RUNNERS[key]


def kernel(x, W_qkv, W_proj):
    in_maps = make_in_maps(x, W_qkv, W_proj)
    runner = _get_runner()
    results = runner.run(in_maps)
    out = np.zeros((B, T, E), dtype=np.float32)
    for c in range(N_CORES):
        out[c // HEADS_PER_CORE] += results[c]["out"]
    return out



# revision 26
# speedup vs baseline: 276.9499x; 1.1356x over previous
"""Causal self-attention (B=2, T=2048, E=2048, H=16, D=128) on 8 TRN2 cores.

Sharding: core c handles batch b = c//4 and head group g = c%4 (4 heads).
 - W_qkv is split column-wise (per head group) -> each core projects only its
   heads' q/k/v. q,k are produced in [d, t] layout, v in [t, d] layout.
 - RoPE is applied on-device in a half-split basis (host permutes W rows for
   q/k so that rotation pairs are (i, i+64) instead of (2i, 2i+1); scores are
   invariant because q and k get the same permutation).
 - scores are computed transposed (S^T tiles [t_k=128, t_q=512]); softmax is
   max-free (scores*scale stay in [-10, 10] for this problem) with the
   denominator accumulated by a ones-matmul on the PE, and the 1/denom scale
   applied to the attention output before the output projection.
 - W_proj is split row-wise; each core emits a partial [T, E] output and the
   host sums the 4 partials per batch.

All matmuls run in bf16 (fp32 PSUM accumulation). Measured end-to-end
absmax-relative error vs the fp32 reference is ~4e-3.

Performance notes (TimelineSim + hardware-loop differencing):
 - wqk is host-permuted to a p-major [p, m, kc, f] DRAM layout so the
   per-m-tile weight DMAs move 4KB-contiguous runs per partition (256B
   descriptors ran at half DMA efficiency): -18us.
 - outp pool depth 4 and denp 3 stop the fused projection's PSUM->SBUF
   copies from backing up the PE during phase-2/3 overlap: -9us.
 - cos/sin load in bf16, split per strip so strip 0's tables land before
   the later wqk m-tiles.
 - the softmax denominator stays on the PE (ones-matmul per exp tile):
   the elementwise engines are ~10x slower per column, so any
   accumulate-elsewhere scheme makes the exp->accumulate chain critical.
 - build_program(cfg={'hw_loop': K}) wraps the body in a device-side
   tc.For_i loop for per-iteration timing by loop-count differencing.
"""

import math
import os

import numpy as np
import ml_dtypes

import concourse.bass as bass
import concourse.mybir as mybir
import concourse.tile as tile
from concourse.vector_clock import ScopedClock

BF16 = mybir.dt.bfloat16
F32 = mybir.dt.float32

B, T, E = 2, 2048, 2048
H, D = 16, 128
N_CORES = 8
HEADS_PER_CORE = 4
KT = 128          # t_k tile (partitions of score tiles)
QS = 512          # t_q strip (free dim of score tiles)
NKC = E // 128    # contraction chunks for the projections
SCALE = 1.0 / math.sqrt(D)

# The walrus build in this container encodes only one sync-wait command per
# instruction (a 3-wait Drain and a 3-wait TensorTensor both fail codegen
# with "Too many sync wait commands"). Tile's scheduler emits multi-wait
# instructions freely, so after scheduling we move each excess wait onto its
# own preceding same-engine NoOp.
_MAX_WAITS = 1


def _split_excess_waits(nc, max_waits=_MAX_WAITS):
    f = nc.m.functions[0]

    def overloaded(ins):
        si = ins.sync_info
        return si is not None and len(si.on_wait) > max_waits

    plan = {}  # bb name -> set of overloaded instruction names
    for bb in f.blocks:
        names = {ins.name for ins in bb.instructions if overloaded(ins)}
        if names:
            plan[bb.name] = names
    if not plan:
        return

    nop_map = {}   # overloaded inst name -> [nop instruction, ...]
    nop_names = set()
    for bb in f.blocks:
        todo = plan.get(bb.name)
        if not todo:
            continue
        for ins in bb.instructions:
            if ins.name not in todo:
                continue
            si = ins.sync_info
            waits = list(si.on_wait)
            excess, keep = waits[:-max_waits], waits[-max_waits:]
            nops = []
            for w in excess:
                nop = nc.engines[ins.engine].nop(nofuse=True).ins
                nop.sync_info = mybir.SyncInfo(on_wait=[w], on_update=[])
                nops.append(nop)
                nop_names.add(nop.name)
            ins.sync_info = mybir.SyncInfo(
                on_wait=keep, on_update=list(si.on_update))
            nop_map[ins.name] = nops

    # The nop builder appended the new instructions to the tail block;
    # remove them from wherever they landed, then splice each in front of
    # its target instruction (same engine => preserves engine order).
    for bb in f.blocks:
        lst = [i for i in bb.instructions if i.name not in nop_names]
        if bb.name in plan:
            out = []
            for i in lst:
                out.extend(nop_map.get(i.name, ()))
                out.append(i)
            lst = out
        bb.instructions = lst


class _TileContext(tile.TileContext):
    def _drain_and_barrier(self, tick_clock, wait_clock):
        nc = self.nc
        drain_inst = nc.sync.drain()
        wait_clock.add_sem_waits(
            drain_inst.ins, ScopedClock({None: tick_clock.global_clock})
        )
        nc.all_engine_barrier()
        assert self.sems is not None
        popped = nc._tile_sem_poison_stack.pop()
        assert popped is self._sem_poison
        nc.clear_and_free_semaphores(list(self.sems.allocated().values()))
        nc.all_engine_barrier()
        _split_excess_waits(nc)


DEFAULT_CFG = dict(
    ps_a=2, ps_s=3, ps_y=2, ps_d=1,
    expp=6, rope=2, xsp=2, outp=4, denp=3,
    denom_acc="pe",     # 'pe': per-tile ones-matmul; 'pool'/'dve': elementwise
                        # accumulate on that engine + one final PE reduce
                        # (measured: elementwise engines are ~10x slower per
                        # column than PE bf16 streaming -- 'pe' wins)
    acc_dt="bf16",      # accumulator dtype for denom_acc pool/dve
    v_copy_engine="scalar",
    exp_pair=False,     # one ACT exp over two score tiles (2-bank PSUM)
    rope_from_sbuf=False,  # crossed RoPE reads direct from PSUM (walrus-legal)
    split_dmas=True,      # chunk weight/x DMAs so first matmuls start early
    split_cs=True,        # cos/sin DMA per strip: strip 0 early, rest late
    cs_bf16=True,         # cos/sin tables in bf16 (frees 8KB/partition)
    y_defer_scale=True,   # copy psy out unscaled; apply 1/denom in place
    tail_scale_pool=False,  # last strip's y copy/scale on Pool (slower: off)
    out_copy_engine="vector",  # PSUM->SBUF copy engine for proj output
    fuse_proj=True,       # emit the proj for each q-strip right after it
    wp_own_slot=True,     # load W_proj into its own tile at kernel start
    hw_loop=1,            # >1: wrap the body in a device-side For_i loop
)


def build_program(cfg=None, n_iters=1):
    cfg = {**DEFAULT_CFG, **(cfg or {})}
    nc = bass.Bass("TRN2", target_bir_lowering=False, debug=False,
                   num_devices=N_CORES)

    xT_d = nc.dram_tensor("xT", [E, T], BF16, kind="ExternalInput")
    # wqk arrives host-permuted as [p, m, kc, f] (p-major) so each m-chunk
    # DMA moves 4KB-contiguous runs per partition instead of 256B ones
    wqk_d = nc.dram_tensor("wqk", [128, 8 * NKC * 128], BF16,
                           kind="ExternalInput")
    wv_d = nc.dram_tensor("wv", [E, 512], BF16, kind="ExternalInput")
    wp_d = nc.dram_tensor("wp", [512, E], BF16, kind="ExternalInput")
    CS = BF16 if cfg["cs_bf16"] else F32
    cos_d = nc.dram_tensor("cos", [128, T], CS, kind="ExternalInput")
    sin_d = nc.dram_tensor("sin", [128, T], CS, kind="ExternalInput")
    out_d = nc.dram_tensor("out", [T, E], F32, kind="ExternalOutput")

    from contextlib import ExitStack

    with _TileContext(nc) as tc, ExitStack() as ctx:
        consts = ctx.enter_context(tc.tile_pool(name="consts", bufs=1))
        wshare = ctx.enter_context(tc.tile_pool(name="wshare", bufs=1))
        xsp = ctx.enter_context(tc.tile_pool(name="xsp", bufs=cfg["xsp"]))
        qkp = ctx.enter_context(tc.tile_pool(name="qkp", bufs=1))
        vp = ctx.enter_context(tc.tile_pool(name="vp", bufs=1))
        yp = ctx.enter_context(tc.tile_pool(name="yp", bufs=1))
        rope = ctx.enter_context(tc.tile_pool(name="rope", bufs=cfg["rope"]))
        expp = ctx.enter_context(tc.tile_pool(name="expp", bufs=cfg["expp"]))
        denp = ctx.enter_context(tc.tile_pool(name="denp", bufs=cfg.get("denp", 2)))
        outp = ctx.enter_context(tc.tile_pool(name="outp", bufs=cfg["outp"]))
        ps_a = ctx.enter_context(
            tc.tile_pool(name="ps_a", bufs=cfg["ps_a"], space="PSUM"))
        ps_s = ctx.enter_context(
            tc.tile_pool(name="ps_s", bufs=cfg["ps_s"], space="PSUM"))
        ps_y = ctx.enter_context(
            tc.tile_pool(name="ps_y", bufs=cfg["ps_y"], space="PSUM"))
        ps_d = ctx.enter_context(
            tc.tile_pool(name="ps_d", bufs=cfg["ps_d"], space="PSUM"))
        accp = None
        if cfg["denom_acc"] != "pe":
            accp = ctx.enter_context(
                tc.tile_pool(name="accp", bufs=cfg.get("accp", 2)))

        dramp = ctx.enter_context(
            tc.tile_pool(name="dramp", bufs=cfg.get("dramp", 4), space="DRAM"))
        if cfg["hw_loop"] > 1:
            ctx.enter_context(tc.For_i(0, cfg["hw_loop"]))
        for _it in range(n_iters):
            # ---- constants / weights ----
            wqk_re = wqk_d.ap().rearrange("p (m kc f) -> p m kc f",
                                          m=8, kc=NKC)
            wqk_sb = wshare.tile([128, 8, NKC, 128], BF16, tag="w")
            cos_sb = consts.tile([128, T], CS)
            sin_sb = consts.tile([128, T], CS)
            wv_sb = consts.tile([128, NKC, 512], BF16)

            def load_xs(s):
                xs = xsp.tile([128, NKC, QS], BF16, tag="xs")
                src = xT_d.ap()[:, s * QS:(s + 1) * QS].rearrange(
                    "(kc p) t -> p kc t", p=128)
                if cfg["split_dmas"] and not cfg.get("xs_batch_late", False):
                    for j in range(4):
                        nc.sync.dma_start(out=xs[:, 4 * j:4 * j + 4, :],
                                          in_=src[:, 4 * j:4 * j + 4, :])
                else:
                    # strips >= 1 are prefetched a full strip ahead; one
                    # large DMA has better descriptor efficiency and frees
                    # the SP sequencer
                    nc.sync.dma_start(out=xs, in_=src)
                return xs

            if cfg["split_dmas"]:
                # loads in exact consumption order: the first M-tile group
                # consumes (wqk[:, kc, 0:128], xs[:, kc, :]) for kc = 0..15,
                # so interleave 4-kc chunks of both streams
                xs_next = xsp.tile([128, NKC, QS], BF16, tag="xs")
                xs0_src = xT_d.ap()[:, 0:QS].rearrange(
                    "(kc p) t -> p kc t", p=128)
                xq = nc.gpsimd if cfg.get("xs0_gpsimd", False) else nc.sync
                for j in range(4):
                    kcs = slice(4 * j, 4 * j + 4)
                    nc.sync.dma_start(out=wqk_sb[:, 0, kcs, :],
                                      in_=wqk_re[:, 0, kcs, :])
                    xq.dma_start(out=xs_next[:, kcs, :],
                                 in_=xs0_src[:, kcs, :])
                n_early = cfg.get("wqk_early", 1)
                for m in range(1, 1 + n_early):
                    nc.sync.dma_start(out=wqk_sb[:, m], in_=wqk_re[:, m])
                if cfg["split_cs"]:
                    # RoPE needs only strip 0's cos/sin early; the rest can
                    # queue behind the remaining wqk m-tiles
                    nc.sync.dma_start(out=cos_sb[:, 0:QS],
                                      in_=cos_d.ap()[:, 0:QS])
                    nc.sync.dma_start(out=sin_sb[:, 0:QS],
                                      in_=sin_d.ap()[:, 0:QS])
                else:
                    nc.sync.dma_start(out=cos_sb, in_=cos_d.ap())
                    nc.sync.dma_start(out=sin_sb, in_=sin_d.ap())
                for m in range(1 + n_early, 8):
                    nc.sync.dma_start(out=wqk_sb[:, m], in_=wqk_re[:, m])
                if cfg["split_cs"]:
                    nc.sync.dma_start(out=cos_sb[:, QS:T],
                                      in_=cos_d.ap()[:, QS:T])
                    nc.sync.dma_start(out=sin_sb[:, QS:T],
                                      in_=sin_d.ap()[:, QS:T])
            else:
                nc.sync.dma_start(out=wqk_sb, in_=wqk_re)
                xs_next = load_xs(0)
                nc.sync.dma_start(out=cos_sb, in_=cos_d.ap())
                nc.sync.dma_start(out=sin_sb, in_=sin_d.ap())
            nc.sync.dma_start(
                out=wv_sb, in_=wv_d.ap().rearrange("(kc p) f -> p kc f", p=128))
            ones_dt = BF16 if (cfg["denom_acc"] == "pe"
                              or cfg["acc_dt"] == "bf16") else F32
            ones_sb = consts.tile([128, 1], ones_dt)
            nc.vector.memset(ones_sb, 1.0)
            # warm the ACT exp table set early so phase 2's first exp
            # doesn't eat the ~2.7us ACT_TABLE_LOAD on the critical path
            warm = consts.tile([128, 1], F32)
            nc.vector.memset(warm, 0.0)
            nc.scalar.activation(warm, warm,
                                 mybir.ActivationFunctionType.Exp)

            qk_rot = qkp.tile([128, 8, T], BF16)   # m<4: q heads, m>=4: k heads
            v_sb = vp.tile([128, T // 128, 512], BF16)  # [t_part, t_tile, 4h*d]
            y_sb = yp.tile([128, HEADS_PER_CORE, T], BF16)  # [d, h, t]

            # ---- phase 1: qkv projection + RoPE ----
            for s in range(T // QS):
                ts = slice(s * QS, (s + 1) * QS)
                xs = xs_next
                if s + 1 < T // QS:
                    xs_next = load_xs(s + 1)
                for m in range(8):
                    # alternate pools: ps_s sits idle during phase 1, so
                    # qkv groups get an effective 5-deep PSUM rotation and
                    # the RoPE-read hold never stalls the next group
                    if cfg.get("p1_pool_mix", False) and m % 2 == 1:
                        ps = ps_s.tile([128, QS], F32, tag="ps_s")
                    else:
                        ps = ps_a.tile([128, QS], F32, tag="ps_a")
                    for kc in range(NKC):
                        nc.tensor.matmul(
                            ps, wqk_sb[:, m, kc, :], xs[:, kc, :],
                            start=(kc == 0), stop=(kc == NKC - 1))
                    # RoPE: rot = q * cos + swap_halves(q) * sin_signed.
                    # walrus rejects TensorTensor with two SBUF inputs at
                    # different base partitions, so the half swap is done by
                    # two SBUF->SBUF DMAs (partition-base offsets are legal
                    # for DMA); all DVE ops are then same-base.
                    if cfg["rope_from_sbuf"] == "act_split":
                        # bank freed after max(ACT straight copy, 2 short
                        # crossed DVE muls from PSUM) ~= 0.8us; the cos
                        # multiply then reads SBUF (same-base, legal)
                        q_sb = rope.tile([128, QS], F32, tag="q_sb")
                        nc.scalar.copy(q_sb, ps)
                        t2 = rope.tile([128, QS], F32, tag="t2")
                        nc.vector.tensor_mul(t2[0:64, :], ps[64:128, :],
                                             sin_sb[0:64, ts])
                        nc.vector.tensor_mul(t2[64:128, :], ps[0:64, :],
                                             sin_sb[64:128, ts])
                        t1 = rope.tile([128, QS], F32, tag="t1")
                        nc.vector.tensor_mul(t1, q_sb, cos_sb[:, ts])
                    elif cfg["rope_from_sbuf"]:
                        # single ACT copy frees the PSUM bank fast
                        q_sb = rope.tile([128, QS], F32, tag="q_sb")
                        nc.scalar.copy(q_sb, ps)
                        qs_sw = rope.tile([128, QS], F32, tag="qs_sw")
                        nc.sync.dma_start(out=qs_sw[0:64, :],
                                          in_=q_sb[64:128, :])
                        nc.sync.dma_start(out=qs_sw[64:128, :],
                                          in_=q_sb[0:64, :])
                        t1 = rope.tile([128, QS], F32, tag="t1")
                        nc.vector.tensor_mul(t1, q_sb, cos_sb[:, ts])
                        t2 = rope.tile([128, QS], F32, tag="t2")
                        nc.vector.tensor_mul(t2, qs_sw, sin_sb[:, ts])
                    else:
                        # PSUM+SBUF operands are exempt from the same-base
                        # rule: crossed reads come straight from PSUM
                        t1 = rope.tile([128, QS], F32, tag="t1")
                        nc.vector.tensor_mul(t1, ps, cos_sb[:, ts])
                        t2 = rope.tile([128, QS], F32, tag="t2")
                        nc.vector.tensor_mul(t2[0:64, :], ps[64:128, :],
                                             sin_sb[0:64, ts])
                        nc.vector.tensor_mul(t2[64:128, :], ps[0:64, :],
                                             sin_sb[64:128, ts])
                    nc.vector.tensor_add(qk_rot[:, m, ts], t1, t2)
                for i in range(QS // 128):
                    tt = 4 * s + i
                    if cfg.get("p1_pool_mix", False) and i % 2 == 1:
                        ps = ps_s.tile([128, 512], F32, tag="ps_s")
                    else:
                        ps = ps_a.tile([128, 512], F32, tag="ps_a")
                    for kc in range(NKC):
                        nc.tensor.matmul(
                            ps, xs[:, kc, i * 128:(i + 1) * 128],
                            wv_sb[:, kc, :],
                            start=(kc == 0), stop=(kc == NKC - 1))
                    last_strip = (s == T // QS - 1
                                  and cfg.get("v_tail_on_dve", True))
                    if cfg["v_copy_engine"] == "scalar" and not last_strip:
                        nc.scalar.copy(v_sb[:, tt, :], ps)
                    else:
                        # keep ACT free at the phase-1 tail so the first
                        # attention exp isn't queued behind these copies
                        nc.vector.tensor_copy(v_sb[:, tt, :], ps)

            # wp in its own slot lets the fused per-strip projection start
            # without waiting for the last wqk read; fall back to sharing
            # the wqk slot if SBUF is tight.
            if cfg["wp_own_slot"]:
                wp_sb = consts.tile([128, 4, E], BF16, tag="wp")
            else:
                wp_sb = wshare.tile([128, 4, E], BF16, tag="w")
            nc.sync.dma_start(
                out=wp_sb, in_=wp_d.ap().rearrange("(ec p) f -> p ec f", p=128))

            # ---- phase 2: attention ----
            from concourse import bass_isa

            def mask_diag(e_ap, qs_i, kt):
                # causal: keep where (tq + qs0) - (tk + kt0) >= 0
                nc.gpsimd.affine_select(
                    out=e_ap, in_=e_ap,
                    compare_op=mybir.AluOpType.is_ge,
                    fill=0.0,
                    base=qs_i * QS - kt * 128,
                    pattern=[[1, QS]],
                    channel_multiplier=-1)

            def proj_tile(ti):
                tsl = slice(ti * 128, (ti + 1) * 128)
                pair = cfg.get("out_dma_pair", False)
                otw = 1024 if pair else 512
                ot = None
                for fs in range(E // 512):
                    ps = ps_a.tile([128, 512], F32, tag="ps_a")
                    for h in range(HEADS_PER_CORE):
                        nc.tensor.matmul(
                            ps, y_sb[:, h, tsl],
                            wp_sb[:, h, fs * 512:(fs + 1) * 512],
                            start=(h == 0), stop=(h == 3))
                    if ot is None:
                        ot = outp.tile([128, otw], F32, tag="ot")
                        o0 = 0
                    oce = cfg["out_copy_engine"]
                    if oce == "alt":
                        oce = "scalar" if fs % 2 else "vector"
                    if oce == "scalar":
                        nc.scalar.copy(ot[:, o0:o0 + 512], ps)
                    elif oce == "pool":
                        nc.gpsimd.tensor_copy(ot[:, o0:o0 + 512], ps)
                    else:
                        nc.vector.tensor_copy(ot[:, o0:o0 + 512], ps)
                    o0 += 512
                    if o0 == otw:
                        f0 = (fs + 1) * 512 - otw
                        nc.sync.dma_start(
                            out=out_d.ap()[tsl, f0:f0 + otw], in_=ot)
                        ot = None

            if cfg["fuse_proj"]:
                # strip 0 first (it only depends on phase-1 strip 0, so the
                # phase-1->2 transition is cheap), then longest-first so the
                # kernel tail is a short strip's attention + proj
                order = cfg.get("strip_order") or [1, 3, 2, 0]
                units = [(h, q) for q in order for h in range(HEADS_PER_CORE)]
                last_q = order[-1]
            else:
                units = [(h, q) for h in range(HEADS_PER_CORE)
                         for q in range(T // QS)]
                last_q = T // QS - 1
            for h, qs_i in units:
                if True:
                    qsl = slice(qs_i * QS, (qs_i + 1) * QS)
                    nk = 4 * qs_i + 4
                    psy = ps_y.tile([128, QS], F32, tag="ps_y")
                    psd = acc = None
                    acc_dt = F32 if cfg["acc_dt"] == "f32" else BF16
                    if cfg["denom_acc"] == "pe":
                        psd = ps_d.tile([1, QS], F32, tag="ps_d")
                    else:
                        acc = accp.tile([128, QS], acc_dt, tag="acc")
                    acc_eng = nc.gpsimd if cfg["denom_acc"] == "pool" \
                        else nc.vector

                    def consume_part(e_ap, kt, d0, w):
                        """denominator + attn@v for the live [128, w] slice
                        of one exp tile (columns d0..QS of the strip)."""
                        if cfg["denom_acc"] == "pe":
                            nc.tensor.matmul(psd[:, d0:d0 + w], ones_sb, e_ap,
                                             start=(kt == 0),
                                             stop=(kt == nk - 1),
                                             skip_group_check=True)
                        else:
                            if kt == 0:
                                acc_eng.tensor_copy(acc, e_ap)
                            else:
                                acc_eng.tensor_add(acc[:, d0:d0 + w],
                                                   acc[:, d0:d0 + w], e_ap)
                        nc.tensor.matmul(psy[:, d0:d0 + w],
                                         v_sb[:, kt, h * 128:(h + 1) * 128],
                                         e_ap, start=(kt == 0),
                                         stop=(kt == nk - 1),
                                         skip_group_check=True)

                    def consume(e_ap, kt):
                        consume_part(e_ap, kt, 0, QS)

                    if not cfg["exp_pair"]:
                        for kt in range(nk):
                            # diagonal tiles: columns tq < d0 are fully
                            # causal-masked, so shrink the score/exp/v work
                            # to the live N = QS - d0 columns
                            d0 = max(0, kt * 128 - qs_i * QS) \
                                if cfg.get("diag_shrink", True) else 0
                            w = QS - d0
                            pss = ps_s.tile([128, QS], F32, tag="ps_s")
                            nc.tensor.matmul(
                                pss[:, 0:w],
                                qk_rot[:, 4 + h, kt * 128:(kt + 1) * 128],
                                qk_rot[:, h,
                                       qs_i * QS + d0:(qs_i + 1) * QS],
                                start=True, stop=True)
                            e = expp.tile([128, QS], BF16, tag="e")
                            nc.scalar.activation(
                                e[:, 0:w], pss[:, 0:w],
                                mybir.ActivationFunctionType.Exp,
                                scale=SCALE)
                            if kt >= 4 * qs_i and w > 1:
                                # keep where local tq index j >= tk
                                nc.gpsimd.affine_select(
                                    out=e[:, 0:w], in_=e[:, 0:w],
                                    compare_op=mybir.AluOpType.is_ge,
                                    fill=0.0, base=0,
                                    pattern=[[1, w]],
                                    channel_multiplier=-1)
                            consume_part(e[:, 0:w], kt, d0, w)
                    else:
                        for kp in range(nk // 2):
                            pss = ps_s.tile([128, 2 * QS], F32, tag="ps_s")
                            for j in range(2):
                                kt = 2 * kp + j
                                nc.tensor.matmul(
                                    pss[:, j * QS:(j + 1) * QS],
                                    qk_rot[:, 4 + h, kt * 128:(kt + 1) * 128],
                                    qk_rot[:, h, qsl], start=True, stop=True)
                            e = expp.tile([128, 2 * QS], BF16, tag="e")
                            nc.scalar.activation(
                                e, pss, mybir.ActivationFunctionType.Exp,
                                scale=SCALE)
                            for j in range(2):
                                kt = 2 * kp + j
                                esl = e[:, j * QS:(j + 1) * QS]
                                if kt >= 4 * qs_i:
                                    mask_diag(esl, qs_i, kt)
                                consume(esl, kt)

                    if cfg["denom_acc"] != "pe":
                        # single partition-reduce of the elementwise
                        # accumulator: psd[0, q] = sum_p acc[p, q]
                        psd = ps_d.tile([1, QS], F32, tag="ps_d")
                        nc.tensor.matmul(psd, ones_sb, acc,
                                         start=True, stop=True)
                    # reciprocal + partition broadcast via DRAM round-trip
                    # (the gpsimd ucode broadcast is unsupported by this
                    # compiler build; DRAM reads may have partition step 0)
                    r = denp.tile([1, QS], F32, tag="r")
                    nc.vector.reciprocal(r, psd)
                    rdram = dramp.tile([1, QS], F32, tag="rd")
                    nc.sync.dma_start(out=rdram, in_=r)
                    rb = denp.tile([128, QS], F32, tag="rb")
                    rbc = bass.AP(tensor=rdram.tensor, offset=rdram.offset,
                                  ap=[[0, 128]] + list(rdram.ap[1:]))
                    nc.sync.dma_start(out=rb, in_=rbc)
                    y_eng = nc.vector
                    if cfg["tail_scale_pool"] and cfg["fuse_proj"] \
                            and qs_i == last_q:
                        # at the kernel tail DVE is the last-proj critical
                        # path; Pool is idle there
                        y_eng = nc.gpsimd
                    if cfg["y_defer_scale"]:
                        # free the psy bank with one copy; the denominator
                        # scale lands later, off the PE critical path
                        y_eng.tensor_copy(y_sb[:, h, qsl], psy)
                        y_eng.tensor_mul(y_sb[:, h, qsl],
                                         y_sb[:, h, qsl], rb)
                    else:
                        y_eng.tensor_mul(y_sb[:, h, qsl], psy, rb)
                if cfg["fuse_proj"] and h == HEADS_PER_CORE - 1:
                    for ti in range(4 * qs_i, 4 * qs_i + 4):
                        proj_tile(ti)

            # ---- phase 3: output projection (partial sums; host reduces) ----
            if not cfg["fuse_proj"]:
                for ti in range(T // 128):
                    proj_tile(ti)

    return nc


_HALF_PERM = np.concatenate([np.arange(0, 128, 2), np.arange(1, 128, 2)])


def make_in_maps(x, W_qkv, W_proj):
    """Host-side sharding: per-core input dict (bf16 where appropriate)."""
    x = np.asarray(x, dtype=np.float32)
    W_qkv = np.asarray(W_qkv, dtype=np.float32)
    W_proj = np.asarray(W_proj, dtype=np.float32)

    t = np.arange(T, dtype=np.float64)
    inv = 10000.0 ** (-np.arange(64, dtype=np.float64) / 64.0)
    ang = t[:, None] * inv[None, :]              # [T, 64]
    cos1 = np.cos(ang).astype(np.float32).T      # [64, T]
    sin1 = np.sin(ang).astype(np.float32).T
    cos_h = np.concatenate([cos1, cos1], axis=0)          # [128, T]
    sin_h = np.concatenate([-sin1, sin1], axis=0)         # [128, T]
    if DEFAULT_CFG["cs_bf16"]:
        cos_h = cos_h.astype(ml_dtypes.bfloat16)
        sin_h = sin_h.astype(ml_dtypes.bfloat16)

    in_maps = []
    for c in range(N_CORES):
        b = c // HEADS_PER_CORE
        g = c % HEADS_PER_CORE
        h0 = g * HEADS_PER_CORE  # first head of this group
        xT = np.ascontiguousarray(x[b].T).astype(ml_dtypes.bfloat16)

        rows_q = []
        rows_k = []
        for h in range(h0, h0 + HEADS_PER_CORE):
            rows_q.append(W_qkv[h * 128 + _HALF_PERM, :])
            rows_k.append(W_qkv[E + h * 128 + _HALF_PERM, :])
        wqk = np.concatenate(rows_q + rows_k, axis=0)          # [1024, E]
        # device layout [p, m, kc, f]: every per-partition run of an m-chunk
        # (and of the interleaved m=0 kc-chunks) is contiguous in DRAM
        wqk = np.ascontiguousarray(
            wqk.reshape(8, 128, NKC, 128).transpose(3, 0, 2, 1)
            .reshape(128, 8 * NKC * 128)).astype(ml_dtypes.bfloat16)
        wv = W_qkv[2 * E + h0 * 128: 2 * E + h0 * 128 + 512, :]
        wv = np.ascontiguousarray(wv.T).astype(ml_dtypes.bfloat16)
        wp = W_proj[:, h0 * 128: h0 * 128 + 512]
        wp = np.ascontiguousarray(wp.T).astype(ml_dtypes.bfloat16)

        in_maps.append({
            "xT": xT, "wqk": wqk, "wv": wv, "wp": wp,
            "cos": cos_h, "sin": sin_h,
        })
    return in_maps


_RUNNERS = {}


def _get_runner(cfg=None, _key=None):
    """Build the Bass program once per cfg and return a cached PJRT runner."""
    key = _key if _key is not None else (
        None if cfg is None else tuple(sorted(cfg.items())))
    if key in _RUNNERS:
        return _RUNNERS[key]

    import jax
    import jax.numpy as jnp  # noqa: F401
    from jax.sharding import Mesh, PartitionSpec
    from jax.experimental.shard_map import shard_map
    from concourse import bass2jax

    nc = build_program(cfg)
    bass2jax.install_neuronx_cc_hook()

    partition_name = (nc.partition_id_tensor.name
                      if nc.partition_id_tensor else None)
    in_names, out_names, out_avals = [], [], []
    for alloc in nc.m.functions[0].allocations:
        if not isinstance(alloc, mybir.MemoryLocationSet):
            continue
        name = alloc.memorylocations[0].name
        if alloc.kind == "ExternalInput":
            if name != partition_name:
                in_names.append(name)
        elif alloc.kind == "ExternalOutput":
            out_names.append(name)
            out_avals.append(jax.core.ShapedArray(
                tuple(alloc.tensor_shape), mybir.dt.np(alloc.dtype)))
    n_params = len(in_names)
    n_outs = len(out_names)
    zero_shapes = [(a.shape, a.dtype) for a in out_avals]
    all_in_names = list(in_names) + list(out_names)
    if partition_name is not None:
        all_in_names.append(partition_name)

    def _body(*args):
        operands = list(args)
        if partition_name is not None:
            operands.append(bass2jax.partition_id_tensor())
        outs = bass2jax._bass_exec_p.bind(
            *operands,
            out_avals=tuple(out_avals),
            in_names=tuple(all_in_names),
            out_names=tuple(out_names),
            lowering_input_output_aliases=(),
            sim_require_finite=True,
            sim_require_nnan=True,
            nc=nc,
        )
        return tuple(outs)

    devices = jax.devices()[:N_CORES]
    mesh = Mesh(np.asarray(devices), ("core",))
    donate = tuple(range(n_params, n_params + n_outs))
    sharded = jax.jit(
        shard_map(_body, mesh=mesh,
                  in_specs=(PartitionSpec("core"),) * (n_params + n_outs),
                  out_specs=(PartitionSpec("core"),) * n_outs,
                  check_rep=False),
        donate_argnums=donate, keep_unused=True)

    class Runner:
        def __init__(self):
            self.sharded = sharded
            self.mesh = mesh
            self.in_names = in_names
            self.out_names = out_names
            self.zero_shapes = zero_shapes

        def prep_inputs(self, in_maps):
            return [
                np.concatenate(
                    [np.asarray(in_maps[c][nm]) for c in range(N_CORES)], axis=0)
                for nm in in_names
            ]

        def zero_set(self):
            return [
                np.zeros((N_CORES * s[0], *s[1:]), d) for (s, d) in zero_shapes
            ]

        def call(self, concat_in, concat_zeros):
            return sharded(*concat_in, *concat_zeros)

        def run(self, in_maps):
            out_arrs = self.call(self.prep_inputs(in_maps), self.zero_set())
            for o in out_arrs:
                o.block_until_ready()
            return [
                {nm: np.asarray(out_arrs[i]).reshape(
                    N_CORES, *zero_shapes[i][0])[c]
                 for i, nm in enumerate(out_names)}
                for c in range(N_CORES)
            ]

    _RUNNERS[key] = Runner()
    return _RUNNERS[key]


def kernel(x, W_qkv, W_proj):
    in_maps = make_in_maps(x, W_qkv, W_proj)
    runner = _get_runner()
    results = runner.run(in_maps)
    out = np.zeros((B, T, E), dtype=np.float32)
    for c in range(N_CORES):
        out[c // HEADS_PER_CORE] += results[c]["out"]
    return out

